# revision 34
# baseline (speedup 1.0000x reference)
"""Trainium2 Bass kernel for an attentive LSTM cell.

Data-parallel across 8 NeuronCores: batch (64) is sharded 8 per core, all
weights replicated.  Annotations are transposed and cast to bf16 on the
host, so the kernel streams ann^T [A, T] tiles that are directly usable as
the uh-matmul moving operand — no on-chip PE transposes, and half the HBM
traffic of fp32.

Per core, per batch row, for each [512, 1024] ann^T tile:
  1. uh^T = ku^T @ ann^T accumulated in PSUM (bf16 matmuls).
  2. tanh(uh + Wx + bias_u) on the scalar engine, N=1024 per instruction
     with the per-partition bias folded in.
  3. et = v . tanh(...) via v-stationary matmuls; exp on the scalar engine
     with the softmax denominator accumulated in the same instruction.
  4. softmax row broadcast to 128 partitions on gpsimd; context computed on
     the vector engine as a fused multiply+reduce over ann^T (contraction
     over t is the free dim in this layout), normalized at the end.

The LSTM tail (z = [x;h] @ [W;R] + b, gates, c/h update) runs batched over
the core's 8 rows with x^T/h^T assembled from tiny PE transposes; W and R
are concatenated and cast to bf16 on the host.
"""

import os
import sys

for _p in ("/opt/trn_rl_repo", "/root/.axon_site/_ro/trn_rl_repo"):
    if os.path.isdir(_p) and _p not in sys.path:
        sys.path.insert(0, _p)

import numpy as np
import ml_dtypes

import concourse.bass as bass
import concourse.mybir as mybir
import concourse.tile as tile
from concourse import bacc
from concourse.bass_utils import run_bass_kernel_spmd
from concourse.masks import make_identity

AF = mybir.ActivationFunctionType
ALU = mybir.AluOpType
F32 = mybir.dt.float32
BF16 = mybir.dt.bfloat16
FP8 = mybir.dt.float8e4
PM = mybir.MatmulPerfMode

UH_FP8 = True    # uh matmul in fp8 DoubleRow (ku prescaled x16 on host)
ET_FP8 = True    # tanh output + et matmul in fp8 DoubleRow
CTX_FP8 = True   # context DVE reduce reads the fp8 annotations (no bf16 copy)
KU_SCALE = 16.0
CTX_ON_POOL = 0  # scalar_tensor_tensor is not a valid Pool-engine op on HW
UHPS_BUFS = 2
ETPS_BUFS = 1
# et = v . tanh(uh + Wx): u-units are sorted by |v| on the host; only the
# EXACT_M top chunks get the tanh (and their uh matmul). For the small-|v|
# rest, tanh(x) ~= x, so their contribution collapses to a precomputed
# rank-1 vector kuv = ku_lin @ v_lin applied directly to ann^T, plus a
# per-row constant folded into the exp bias.
EXACT_M = 2

N_CORES = 8
B, T, A, U, D = 64, 2048, 512, 512, 512
BS = B // N_CORES   # batch rows per core
TT = 1024           # t macro-tile
NT = T // TT        # macro tiles per batch row
TH = TT // 512      # 512-col halves per macro tile (PSUM bank granularity)
J = A // 128        # contraction chunks (annotation dim)
M = U // 128        # unit chunks
KZ = (D + A + U) // 128  # contraction chunks for the z matmul ([x; h])


def build_bass(stage="full", repeat=1):
    nc = bacc.Bacc(trn_type="TRN2", debug=False)

    AT = FP8 if UH_FP8 else BF16
    EU = 128 * EXACT_M
    annT_d = nc.dram_tensor("annT", [BS, A, T], AT, kind="ExternalInput").ap()
    inp_d = nc.dram_tensor("inputs", [BS, D], F32, kind="ExternalInput").ap()
    h_d = nc.dram_tensor("h", [BS, U], F32, kind="ExternalInput").ap()
    c_d = nc.dram_tensor("c", [BS, U], F32, kind="ExternalInput").ap()
    WR_d = nc.dram_tensor("wr", [D + A + U, 4 * U], BF16, kind="ExternalInput").ap()
    bias_d = nc.dram_tensor("bias", [1, 6 * U], F32, kind="ExternalInput").ap()
    ku_d = nc.dram_tensor("ku", [A, EU], AT, kind="ExternalInput").ap()
    kw_d = nc.dram_tensor("kw", [U, U], BF16, kind="ExternalInput").ap()
    kv_d = nc.dram_tensor("kv", [1, EU], FP8 if ET_FP8 else BF16,
                          kind="ExternalInput").ap()
    out_d = nc.dram_tensor("out", [BS, U], F32, kind="ExternalOutput").ap()
    bup_d = nc.dram_tensor("bup", [1, U], F32, kind="ExternalInput").ap()
    kuv_d = None
    kvf_d = None
    if EXACT_M < M:
        kuv_d = nc.dram_tensor("kuv", [A, 1], AT, kind="ExternalInput").ap()
        kvf_d = nc.dram_tensor("kvf", [1, U], BF16, kind="ExternalInput").ap()
    annC_d = None
    if not CTX_FP8 and UH_FP8:
        annC_d = nc.dram_tensor("annC", [BS, A, T], BF16,
                                kind="ExternalInput").ap()

    dd = dict(annT_d=annT_d, inp_d=inp_d, h_d=h_d, c_d=c_d, WR_d=WR_d,
              bias_d=bias_d, ku_d=ku_d, kw_d=kw_d, kv_d=kv_d, out_d=out_d,
              bup_d=bup_d, kuv_d=kuv_d, kvf_d=kvf_d, annC_d=annC_d)
    with tile.TileContext(nc) as tc:
        if repeat > 1:
            with tc.For_i(0, repeat, 1):
                _body(nc, tc, stage=stage, **dd)
        else:
            _body(nc, tc, stage=stage, **dd)
    nc.compile()
    return nc


def _body(nc, tc, annT_d, inp_d, h_d, c_d, WR_d, bias_d, ku_d, kw_d, kv_d,
          out_d, bup_d, kuv_d=None, kvf_d=None, annC_d=None, stage="full"):
    AT = FP8 if UH_FP8 else BF16
    ET = FP8 if ET_FP8 else BF16
    EU = 128 * EXACT_M
    with (
        tc.tile_pool(name="const", bufs=1) as cpool,
        tc.tile_pool(name="wts", bufs=1) as wpool,
    ):
        # touch the activation table set (exp/tanh share one) at t=0 so the
        # ~2.7us LoadActFuncSet runs before any real dependency chain
        warm = cpool.tile([1, 2], F32)
        nc.vector.memset(warm, 0.0)
        nc.scalar.activation(warm[:, 1:2], warm[:, 0:1], AF.Exp)

        ident = cpool.tile([128, 128], F32)
        make_identity(nc, ident)
        ones1b_ld = cpool.tile([1, BS], F32)
        nc.vector.memset(ones1b_ld, 1.0)
        ones1b = cpool.tile([1, BS], BF16)
        nc.vector.tensor_copy(ones1b, ones1b_ld)
        half_col = cpool.tile([BS, 1], F32)
        nc.vector.memset(half_col, 0.5)

        # --- replicated weights (already quantized in DRAM) ---
        ku_sb = wpool.tile([128, J, EU], AT)    # ku[a, u] -> [p, j, u]
        nc.sync.dma_start(out=ku_sb, in_=ku_d.rearrange("(j p) u -> p j u", p=128))
        kw_sb = wpool.tile([128, J, U], BF16)
        nc.sync.dma_start(out=kw_sb, in_=kw_d.rearrange("(j p) u -> p j u", p=128))
        if kuv_d is not None:
            # rank-1 linear-tanh weights; 16B-padded for DoubleRow LDWEIGHTS
            kuv_sb = wpool.tile([128, J, 16], AT)
            nc.sync.dma_start(
                out=kuv_sb[:, :, 0:1],
                in_=kuv_d.rearrange("(j p) o -> p j o", p=128))
            kvf_col = wpool.tile([128, M], BF16)   # full |v|-sorted v
            nc.sync.dma_start(
                out=kvf_col, in_=kvf_d.rearrange("o (m p) -> p (o m)", p=128))
        # z-matmul weights: all four gates resident up front (DMA triggered
        # at b==1 so the b=0 annotation tiles are queued first)
        Wn_t = [wpool.tile([128, KZ, U], BF16, tag=f"wn{n}", name=f"wn{n}")
                for n in range(4)]
        z_part = wpool.tile([BS, 4, U], F32)      # x/h part of z, gate 2 raw
        z_part_sc = wpool.tile([BS, 4, U], F32)   # 0.2*z_part+0.5 for i/f/o
        # v[u] -> [p, m, 0]; 16B-padded m-stride (fp8 DoubleRow LDWEIGHTS
        # requires the Ko-group step to be a multiple of 16 bytes)
        v_col = cpool.tile([128, EXACT_M, 16], ET)
        nc.sync.dma_start(
            out=v_col[:, :, 0:1], in_=kv_d.rearrange("o (m p) -> p m o", p=128))
        biasu_col = cpool.tile([128, M], F32)   # |v|-sorted bias_u column
        nc.sync.dma_start(
            out=biasu_col, in_=bup_d.rearrange("o (m p) -> p (o m)", p=128))
        biasz_ld = cpool.tile([1, 4 * U], F32)
        nc.sync.dma_start(out=biasz_ld, in_=bias_d[:, 0:4 * U])
        biasz_row = cpool.tile([1, 4 * U], BF16)
        nc.vector.tensor_copy(biasz_row, biasz_ld)

        # --- per-core state rows ---
        h_nat = cpool.tile([BS, U], F32)
        nc.sync.dma_start(out=h_nat, in_=h_d)
        in_nat = cpool.tile([BS, D], F32)
        nc.sync.dma_start(out=in_nat, in_=inp_d)
        c_nat = cpool.tile([BS, U], F32)
        nc.sync.dma_start(out=c_nat, in_=c_d)

        # [inputs; context; h]^T in contraction layout, bf16 for the z matmul
        xhT = wpool.tile([128, KZ, BS], BF16)
        bias_att = wpool.tile([128, M, BS], F32)  # Wx^T + bias_u per batch row

        with tc.tile_pool(name="ps_setup", bufs=2, space="PSUM") as pps:
            for j in range(J):
                pt = pps.tile([128, BS], F32)
                nc.tensor.transpose(pt, in_nat[:, 128 * j:128 * (j + 1)],
                                    ident[0:BS, 0:BS])
                nc.vector.tensor_copy(xhT[:, j, :], pt)
            for j in range(M):
                pt = pps.tile([128, BS], F32)
                nc.tensor.transpose(pt, h_nat[:, 128 * j:128 * (j + 1)],
                                    ident[0:BS, 0:BS])
                nc.vector.tensor_copy(xhT[:, 2 * J + j, :], pt)
            for m in range(M):
                pwx = pps.tile([128, BS], F32)
                for j in range(M):
                    nc.tensor.matmul(pwx,
                                     lhsT=kw_sb[:, j, 128 * m:128 * (m + 1)],
                                     rhs=xhT[:, 2 * J + j, :],
                                     start=(j == 0), stop=(j == M - 1))
                nc.scalar.activation(bias_att[:, m, :], pwx, AF.Identity,
                                     bias=biasu_col[:, m:m + 1])
            if kuv_d is not None:
                # per-row constant of the linearized-tanh part:
                # c0[b] = sum_lin v_u * (Wx + bias_u)[u, b]
                ba16 = wpool.tile([128, M - EXACT_M, BS], BF16)
                nc.vector.tensor_copy(ba16, bias_att[:, EXACT_M:, :])
                c0_ps = pps.tile([1, BS], F32, name="c0ps")
                for idx, m in enumerate(range(EXACT_M, M)):
                    nc.tensor.matmul(c0_ps, lhsT=kvf_col[:, m:m + 1],
                                     rhs=ba16[:, idx, :],
                                     start=(idx == 0),
                                     stop=(idx == M - EXACT_M - 1))
                c0_row = wpool.tile([1, BS], F32)
                nc.vector.tensor_copy(c0_row, c0_ps)

        dump = cpool.tile([BS, U], F32)
        nc.vector.memset(dump, 0.0)
        if stage == "setup":
            nc.vector.tensor_copy(dump[:, 0:BS], xhT[0:BS, 0, :])
            nc.vector.tensor_copy(dump[:, BS:2 * BS], bias_att[0:BS, 0, :])
            nc.sync.dma_start(out=out_d, in_=dump)
            return

        # ------------- attention over the annotation stream -------------
        with (
            tc.tile_pool(name="ann", bufs=4) as annpool,
            tc.tile_pool(name="tanh", bufs=2) as tanhpool,
            tc.tile_pool(name="uh_ps", bufs=UHPS_BUFS, space="PSUM") as uhps,
            tc.tile_pool(name="et_ps", bufs=ETPS_BUFS, space="PSUM") as etps,
            tc.tile_pool(name="zp_ps", bufs=1, space="PSUM") as zpps,
            tc.tile_pool(name="small_sb", bufs=2) as smallsb,
            tc.tile_pool(name="wb_sb", bufs=2) as wbpool,
            tc.tile_pool(name="scr_sb", bufs=2) as scrpool,
        ):
            for b in range(BS):
                # x/h partial of one z-gate per odd b: overlaps the LSTM
                # matmuls with attention, leaving only ctx@W on the tail
                if stage == "full" and 1 <= b <= 4:
                    n = b - 1
                    nc.sync.dma_start(
                        out=Wn_t[n],
                        in_=WR_d[:, U * n:U * (n + 1)].rearrange(
                            "(k p) n -> p k n", p=128))
                if stage == "full" and b % 2 == 1:
                    n = (b - 1) // 2
                    zp = zpps.tile([BS, U], F32, tag="zp")
                    for idx, k in enumerate([0, 1, 2, 3, 8, 9, 10, 11]):
                        nc.tensor.matmul(zp, lhsT=xhT[:, k, :],
                                         rhs=Wn_t[n][:, k, :],
                                         start=(idx == 0), stop=False)
                    nc.tensor.matmul(zp, lhsT=ones1b,
                                     rhs=biasz_row[:, U * n:U * (n + 1)],
                                     start=False, stop=True)
                    if n == 2:
                        nc.vector.tensor_copy(z_part[:, n, :], zp)
                    else:
                        nc.vector.tensor_scalar(z_part_sc[:, n, :], zp,
                                                0.2, 0.5, op0=ALU.mult,
                                                op1=ALU.add)

                ctx_slot = smallsb.tile([128, J, NT], F32, tag="ctxslot")
                denb = smallsb.tile([1, NT], F32, tag="den")
                for i in range(NT):
                    annT = annpool.tile([128, J, TT], AT)
                    nc.sync.dma_start(
                        out=annT,
                        in_=annT_d[b, :, TT * i:TT * (i + 1)].rearrange(
                            "(j p) t -> p j t", p=128))
                    if annC_d is not None:
                        annC = annpool.tile([128, J, TT], BF16, tag="annC")
                        nc.sync.dma_start(
                            out=annC,
                            in_=annC_d[b, :, TT * i:TT * (i + 1)].rearrange(
                                "(j p) t -> p j t", p=128))
                    else:
                        annC = annT

                    # uh^T = ku^T @ ann^T, tanh(+Wx+bias_u) per exact m-chunk
                    tanhG = tanhpool.tile([128, EXACT_M, TT], ET)
                    for m in range(EXACT_M):
                        gps = uhps.tile([128, TH, 512], F32, tag="uh")
                        for th in range(TH):
                            if UH_FP8:
                                for g in range(J // 2):
                                    nc.tensor.matmul(
                                        gps[:, th, :],
                                        lhsT=ku_sb[:, 2 * g:2 * g + 2,
                                                   128 * m:128 * (m + 1)],
                                        rhs=annT[:, 2 * g:2 * g + 2,
                                                 512 * th:512 * (th + 1)],
                                        start=(g == 0), stop=(g == J // 2 - 1),
                                        perf_mode=PM.DoubleRow)
                            else:
                                for j in range(J):
                                    nc.tensor.matmul(
                                        gps[:, th, :],
                                        lhsT=ku_sb[:, j, 128 * m:128 * (m + 1)],
                                        rhs=annT[:, j, 512 * th:512 * (th + 1)],
                                        start=(j == 0), stop=(j == J - 1))
                        nc.scalar.activation(tanhG[:, m, :], gps, AF.Tanh,
                                             bias=bias_att[:, m, b:b + 1],
                                             scale=(1.0 / KU_SCALE
                                                    if UH_FP8 else 1.0))
                    if stage == "g":
                        nc.vector.tensor_copy(dump, tanhG[0:BS, 0, 0:U])
                        continue

                    # et = v . tanhG + kuv . ann^T  (PE), exp + den (scalar)
                    et_ps = etps.tile([1, TH, 512], F32, tag="et")
                    for th in range(TH):
                        ts = slice(512 * th, 512 * (th + 1))
                        mms = []
                        if ET_FP8:
                            for g in range(EXACT_M // 2):
                                mms.append((v_col[:, 2 * g:2 * g + 2, 0:1],
                                            tanhG[:, 2 * g:2 * g + 2, ts],
                                            PM.DoubleRow))
                        else:
                            for m in range(EXACT_M):
                                mms.append((v_col[:, m, 0:1],
                                            tanhG[:, m, ts], None))
                        if kuv_d is not None:
                            for g in range(J // 2):
                                mms.append((kuv_sb[:, 2 * g:2 * g + 2, 0:1],
                                            annT[:, 2 * g:2 * g + 2, ts],
                                            PM.DoubleRow))
                        for q, (lt, rh, pm) in enumerate(mms):
                            nc.tensor.matmul(et_ps[:, th, :], lhsT=lt, rhs=rh,
                                             start=(q == 0),
                                             stop=(q == len(mms) - 1),
                                             perf_mode=pm)
                    w_row = smallsb.tile([1, TT], BF16, tag="wrow")
                    nc.scalar.activation(w_row, et_ps, AF.Exp,
                                         scale=(1.0 / KU_SCALE
                                                if ET_FP8 else 1.0),
                                         bias=(c0_row[:, b:b + 1]
                                               if kuv_d is not None else 0.0),
                                         accum_out=denb[:, i:i + 1])
                    if stage == "et":
                        nc.vector.tensor_copy(dump[0:1, 0:U], w_row[:, 0:U])
                        continue

                    # context: broadcast softmax row, fused mul+reduce on DVE
                    wb = wbpool.tile([128, TT], BF16)
                    nc.gpsimd.partition_broadcast(wb, w_row)
                    scr = scrpool.tile([128, TT], BF16)
                    scr2 = scrpool.tile([128, TT], BF16, tag="scr2")
                    for j in range(J):
                        eng = (nc.gpsimd if j < CTX_ON_POOL else nc.vector)
                        eng.scalar_tensor_tensor(
                            out=(scr2 if j < CTX_ON_POOL else scr),
                            in0=annC[:, j, :], scalar=1.0, in1=wb,
                            op0=ALU.mult, op1=ALU.mult,
                            accum_out=ctx_slot[:, j, i:i + 1])

                if stage in ("g", "et"):
                    continue
                # normalize context into xhT[:, J:2J, b]
                dsum = smallsb.tile([1, 1], F32, tag="dsum")
                nc.vector.reduce_sum(dsum, denb, axis=mybir.AxisListType.X)
                drec = smallsb.tile([1, 1], F32, tag="drec")
                nc.vector.reciprocal(drec, dsum)
                drec_b = smallsb.tile([128, 1], F32, tag="drecb")
                nc.gpsimd.partition_broadcast(drec_b, drec)
                ctx_sum = smallsb.tile([128, J], F32, tag="ctxsum")
                nc.vector.tensor_add(ctx_sum, ctx_slot[:, :, 0],
                                     ctx_slot[:, :, 1])
                nc.vector.tensor_scalar_mul(xhT[:, J:2 * J, b:b + 1],
                                            ctx_sum, drec_b)
                if stage == "ctx":
                    nc.vector.tensor_copy(dump[0:1, 0:J], ctx_sum[0:1, :])

        if stage in ("g", "et", "ctx"):
            nc.sync.dma_start(out=out_d, in_=dump)
            return

        # ------------- LSTM tail: only ctx @ W remains here -------------
        with (
            tc.tile_pool(name="z_ps", bufs=1, space="PSUM") as zpool,
            tc.tile_pool(name="gates", bufs=1) as gpool,
        ):
            gates = [None] * 4
            for n in (2, 0, 1, 3):  # tanh gate first: ACT overlaps later MMs
                zps = zpool.tile([BS, U], F32, tag=f"zps{n}")
                for idx, k in enumerate(range(J, 2 * J)):
                    nc.tensor.matmul(zps, lhsT=xhT[:, k, :],
                                     rhs=Wn_t[n][:, k, :],
                                     start=(idx == 0), stop=(idx == J - 1))
                g = gpool.tile([BS, U], F32, tag=f"gate{n}")
                if n == 2:  # candidate cell state: tanh(z)
                    zsb = gpool.tile([BS, U], F32, tag="zsb2")
                    nc.vector.scalar_tensor_tensor(
                        out=zsb, in0=zps, scalar=0.0, in1=z_part[:, n, :],
                        op0=ALU.add, op1=ALU.add)
                    nc.scalar.activation(g, zsb, AF.Tanh)
                else:       # hard sigmoid: clip(0.2 z + 0.5, 0, 1) on DVE
                    zsb = gpool.tile([BS, U], F32, tag=f"zsb{n}")
                    nc.vector.scalar_tensor_tensor(
                        out=zsb, in0=zps, scalar=0.2,
                        in1=z_part_sc[:, n, :], op0=ALU.mult, op1=ALU.add)
                    nc.vector.tensor_scalar(g, zsb, 0.0, 1.0,
                                            op0=ALU.max, op1=ALU.min)
                gates[n] = g

            gi, gf, gg, go = gates
            c_new = gpool.tile([BS, U], F32, tag="cnew")
            nc.vector.tensor_mul(c_new, gf, c_nat)
            ig = gpool.tile([BS, U], F32, tag="ig")
            nc.vector.tensor_mul(ig, gi, gg)
            nc.vector.tensor_add(c_new, c_new, ig)
            tc_t = gpool.tile([BS, U], F32, tag="tanhc")
            nc.scalar.activation(tc_t, c_new, AF.Tanh)
            h_new = gpool.tile([BS, U], F32, tag="hnew")
            nc.vector.tensor_mul(h_new, go, tc_t)
            nc.sync.dma_start(out=out_d, in_=h_new)


_NC_CACHE = None


def _get_nc():
    global _NC_CACHE
    if _NC_CACHE is None:
        _NC_CACHE = build_bass()
    return _NC_CACHE


def make_in_maps(inputs, h, c, annotations, kernel, recurrent_kernel, bias,
                 kernel_u, kernel_w, kernel_v):
    asc = np.ascontiguousarray
    bf = ml_dtypes.bfloat16
    f8 = ml_dtypes.float8_e4m3
    at = f8 if UH_FP8 else bf
    wr = np.concatenate([np.asarray(kernel, np.float32),
                         np.asarray(recurrent_kernel, np.float32)],
                        axis=0).astype(bf)
    bias_f = asc(np.asarray(bias, np.float32)).reshape(1, 6 * U)
    # sort attention units by |v| so the small-|v| tail can use tanh(x)~=x
    kv_f = np.asarray(kernel_v, np.float32)
    perm = np.argsort(-np.abs(kv_f))
    EUv = 128 * EXACT_M
    ku_f = np.asarray(kernel_u, np.float32)[:, perm]
    kw_f = np.asarray(kernel_w, np.float32)[:, perm]
    kv_p = kv_f[perm]
    bup = asc(bias_f[0, 4 * U:5 * U][perm]).reshape(1, U)
    ku_q = (asc((ku_f[:, :EUv] * KU_SCALE).astype(f8)) if UH_FP8
            else asc(ku_f[:, :EUv].astype(bf)))
    kw16 = asc(kw_f.astype(bf))
    kv_q = (asc((kv_p[:EUv] * KU_SCALE).astype(f8)) if ET_FP8
            else asc(kv_p[:EUv].astype(bf))).reshape(1, EUv)
    lin_scale = KU_SCALE if ET_FP8 else 1.0
    kuv = asc((lin_scale * (ku_f[:, EUv:] @ kv_p[EUv:]))
              .astype(f8 if UH_FP8 else bf)).reshape(A, 1)
    kvf = asc(kv_p.astype(bf)).reshape(1, U)
    maps = []
    for core in range(N_CORES):
        sl = slice(core * BS, (core + 1) * BS)
        annT_f = np.ascontiguousarray(
            np.asarray(annotations[sl], np.float32).transpose(0, 2, 1))
        m = {
            "annT": annT_f.astype(at),
            "inputs": asc(inputs[sl]).astype(np.float32),
            "h": asc(h[sl]).astype(np.float32),
            "c": asc(c[sl]).astype(np.float32),
            "wr": wr,
            "bias": bias_f,
            "ku": ku_q,
            "kw": kw16,
            "kv": kv_q,
            "bup": bup,
        }
        if EXACT_M < M:
            m["kuv"] = kuv
            m["kvf"] = kvf
        if UH_FP8 and not CTX_FP8:
            m["annC"] = annT_f.astype(bf)
        maps.append(m)
    return maps


def kernel(inputs, h, c, annotations, kernel, recurrent_kernel, bias,
           kernel_u, kernel_w, kernel_v, _trace=False):
    nc = _get_nc()
    in_maps = make_in_maps(inputs, h, c, annotations, kernel,
                           recurrent_kernel, bias, kernel_u, kernel_w,
                           kernel_v)
    res = run_bass_kernel_spmd(nc, in_maps, list(range(N_CORES)),
                               trace=_trace)
    out = np.concatenate([res.results[i]["out"] for i in range(N_CORES)],
                         axis=0)
    if _trace:
        kernel.last_exec_time_ns = res.exec_time_ns
        kernel.last_results = res
    return out


# revision 40
# speedup vs baseline: 1.2218x; 1.2218x over previous
"""Trainium2 Bass kernel for an attentive LSTM cell.

Data-parallel across 8 NeuronCores: batch (64) is sharded 8 per core, all
weights replicated.  Annotations are transposed and cast to bf16 on the
host, so the kernel streams ann^T [A, T] tiles that are directly usable as
the uh-matmul moving operand — no on-chip PE transposes, and half the HBM
traffic of fp32.

Per core, per batch row, for each [512, 1024] ann^T tile:
  1. uh^T = ku^T @ ann^T accumulated in PSUM (bf16 matmuls).
  2. tanh(uh + Wx + bias_u) on the scalar engine, N=1024 per instruction
     with the per-partition bias folded in.
  3. et = v . tanh(...) via v-stationary matmuls; exp on the scalar engine
     with the softmax denominator accumulated in the same instruction.
  4. softmax row broadcast to 128 partitions on gpsimd; context computed on
     the vector engine as a fused multiply+reduce over ann^T (contraction
     over t is the free dim in this layout), normalized at the end.

The LSTM tail (z = [x;h] @ [W;R] + b, gates, c/h update) runs batched over
the core's 8 rows with x^T/h^T assembled from tiny PE transposes; W and R
are concatenated and cast to bf16 on the host.
"""

import os
import sys

for _p in ("/opt/trn_rl_repo", "/root/.axon_site/_ro/trn_rl_repo"):
    if os.path.isdir(_p) and _p not in sys.path:
        sys.path.insert(0, _p)

import numpy as np
import ml_dtypes

import concourse.bass as bass
import concourse.mybir as mybir
import concourse.tile as tile
from concourse import bacc
from concourse.bass_utils import run_bass_kernel_spmd
from concourse.masks import make_identity

AF = mybir.ActivationFunctionType
ALU = mybir.AluOpType
F32 = mybir.dt.float32
BF16 = mybir.dt.bfloat16
FP8 = mybir.dt.float8e4
PM = mybir.MatmulPerfMode

UH_FP8 = True    # uh matmul in fp8 DoubleRow (ku prescaled x16 on host)
ET_FP8 = True    # tanh output + et matmul in fp8 DoubleRow
CTX_FP8 = True   # context DVE reduce reads the fp8 annotations (no bf16 copy)
KU_SCALE = 16.0
CTX_ON_POOL = 0  # scalar_tensor_tensor is not a valid Pool-engine op on HW
UHPS_BUFS = 2
ETPS_BUFS = 1
# et = v . tanh(uh + Wx): u-units are sorted by |v| on the host; only the
# EXACT_M top chunks get the tanh (and their uh matmul). For the small-|v|
# rest, tanh(x) ~= x, so their contribution collapses to a precomputed
# rank-1 vector kuv = ku_lin @ v_lin applied directly to ann^T, plus a
# per-row constant folded into the exp bias.
EXACT_M = 2

N_CORES = 8
B, T, A, U, D = 64, 2048, 512, 512, 512
BS = B // N_CORES   # batch rows per core
TT = 1024           # t macro-tile
NT = T // TT        # macro tiles per batch row
TH = TT // 512      # 512-col halves per macro tile (PSUM bank granularity)
J = A // 128        # contraction chunks (annotation dim)
M = U // 128        # unit chunks
KZ = (D + A + U) // 128  # contraction chunks for the z matmul ([x; h])


def build_bass(stage="full", repeat=1):
    nc = bacc.Bacc(trn_type="TRN2", debug=False)

    AT = FP8 if UH_FP8 else BF16
    EU = 128 * EXACT_M
    annT_d = nc.dram_tensor("annT", [BS, A, T], AT, kind="ExternalInput").ap()
    inp_d = nc.dram_tensor("inputs", [BS, D], F32, kind="ExternalInput").ap()
    h_d = nc.dram_tensor("h", [BS, U], F32, kind="ExternalInput").ap()
    c_d = nc.dram_tensor("c", [BS, U], F32, kind="ExternalInput").ap()
    WR_d = nc.dram_tensor("wr", [D + A + U, 4 * U], BF16, kind="ExternalInput").ap()
    bias_d = nc.dram_tensor("bias", [1, 6 * U], F32, kind="ExternalInput").ap()
    ku_d = nc.dram_tensor("ku", [A, EU], AT, kind="ExternalInput").ap()
    kw_d = nc.dram_tensor("kw", [U, U], BF16, kind="ExternalInput").ap()
    kv_d = nc.dram_tensor("kv", [1, EU], F32, kind="ExternalInput").ap()
    out_d = nc.dram_tensor("out", [BS, U], F32, kind="ExternalOutput").ap()
    bup_d = nc.dram_tensor("bup", [1, U], F32, kind="ExternalInput").ap()
    kuv_d = None
    kvf_d = None
    if EXACT_M < M:
        kuv_d = nc.dram_tensor("kuv", [1, A], F32, kind="ExternalInput").ap()
        kvf_d = nc.dram_tensor("kvf", [1, U], F32, kind="ExternalInput").ap()
    annC_d = None
    if not CTX_FP8 and UH_FP8:
        annC_d = nc.dram_tensor("annC", [BS, A, T], BF16,
                                kind="ExternalInput").ap()

    dd = dict(annT_d=annT_d, inp_d=inp_d, h_d=h_d, c_d=c_d, WR_d=WR_d,
              bias_d=bias_d, ku_d=ku_d, kw_d=kw_d, kv_d=kv_d, out_d=out_d,
              bup_d=bup_d, kuv_d=kuv_d, kvf_d=kvf_d, annC_d=annC_d)
    with tile.TileContext(nc) as tc:
        if repeat > 1:
            with tc.For_i(0, repeat, 1):
                _body(nc, tc, stage=stage, **dd)
        else:
            _body(nc, tc, stage=stage, **dd)
    nc.compile()
    return nc


def _body(nc, tc, annT_d, inp_d, h_d, c_d, WR_d, bias_d, ku_d, kw_d, kv_d,
          out_d, bup_d, kuv_d=None, kvf_d=None, annC_d=None, stage="full"):
    AT = FP8 if UH_FP8 else BF16
    ET = FP8 if ET_FP8 else BF16
    EU = 128 * EXACT_M
    with (
        tc.tile_pool(name="const", bufs=1) as cpool,
        tc.tile_pool(name="wts", bufs=1) as wpool,
    ):
        # touch the activation table set (exp/tanh share one) at t=0 so the
        # ~2.7us LoadActFuncSet runs before any real dependency chain
        warm = cpool.tile([1, 2], F32)
        nc.vector.memset(warm, 0.0)
        nc.scalar.activation(warm[:, 1:2], warm[:, 0:1], AF.Exp)

        ident = cpool.tile([128, 128], F32)
        make_identity(nc, ident)
        ones11 = cpool.tile([1, 1], F32)
        nc.vector.memset(ones11, 1.0)
        ones1b_ld = cpool.tile([1, BS], F32)
        nc.vector.memset(ones1b_ld, 1.0)
        ones1b = cpool.tile([1, BS], BF16)
        nc.vector.tensor_copy(ones1b, ones1b_ld)
        half_col = cpool.tile([BS, 1], F32)
        nc.vector.memset(half_col, 0.5)

        # --- replicated weights (already quantized in DRAM) ---
        ku_sb = wpool.tile([128, J, EU], AT)    # ku[a, u] -> [p, j, u]
        nc.sync.dma_start(out=ku_sb, in_=ku_d.rearrange("(j p) u -> p j u", p=128))
        kw_sb = wpool.tile([128, J, U], BF16)
        nc.sync.dma_start(out=kw_sb, in_=kw_d.rearrange("(j p) u -> p j u", p=128))
        # Small vectors are DMA'd as single contiguous rows and spread to
        # per-partition columns with tiny PE transposes: a rearrange-gather
        # DMA here would cost hundreds of 1-4 byte descriptors per repeat.
        if kuv_d is not None:
            # rank-1 linear-tanh weights; 16B-padded for DoubleRow LDWEIGHTS
            kuv_sb = wpool.tile([128, J, 16], AT)
            kuv_row = cpool.tile([1, A], F32)
            nc.sync.dma_start(out=kuv_row, in_=kuv_d)
            kvf_col = wpool.tile([128, M], BF16)   # full |v|-sorted v
            kvf_row = cpool.tile([1, U], F32)
            nc.sync.dma_start(out=kvf_row, in_=kvf_d)
        # z-matmul weights: all four gates resident up front (DMA triggered
        # at b==1 so the b=0 annotation tiles are queued first)
        Wn_t = [wpool.tile([128, KZ, U], BF16, tag=f"wn{n}", name=f"wn{n}")
                for n in range(4)]
        z_part = wpool.tile([BS, 4, U], F32)      # x/h part of z, gate 2 raw
        z_part_sc = wpool.tile([BS, 4, U], F32)   # 0.2*z_part+0.5 for i/f/o
        # v[u] -> [p, m, 0]; 16B-padded m-stride (fp8 DoubleRow LDWEIGHTS
        # requires the Ko-group step to be a multiple of 16 bytes)
        v_col = cpool.tile([128, EXACT_M, 16], ET)
        kv_row = cpool.tile([1, EU], F32)
        nc.sync.dma_start(out=kv_row, in_=kv_d)
        biasu_col = cpool.tile([128, M], F32)   # |v|-sorted bias_u column
        bup_row = cpool.tile([1, U], F32)
        nc.sync.dma_start(out=bup_row, in_=bup_d)
        biasz_ld = cpool.tile([1, 4 * U], F32)
        nc.sync.dma_start(out=biasz_ld, in_=bias_d[:, 0:4 * U])
        biasz_row = cpool.tile([1, 4 * U], BF16)
        nc.vector.tensor_copy(biasz_row, biasz_ld)

        # --- per-core state rows ---
        h_nat = cpool.tile([BS, U], F32)
        nc.sync.dma_start(out=h_nat, in_=h_d)
        in_nat = cpool.tile([BS, D], F32)
        nc.sync.dma_start(out=in_nat, in_=inp_d)
        c_nat = cpool.tile([BS, U], F32)
        nc.sync.dma_start(out=c_nat, in_=c_d)

        # [inputs; context; h]^T in contraction layout, bf16 for the z matmul
        xhT = wpool.tile([128, KZ, BS], BF16)
        bias_att = wpool.tile([128, M, BS], F32)  # Wx^T + bias_u per batch row

        with tc.tile_pool(name="ps_setup", bufs=2, space="PSUM") as pps:
            # spread the row-loaded vectors into per-partition columns
            fills = [(biasu_col[:, m:m + 1], bup_row[:, 128 * m:128 * (m + 1)])
                     for m in range(M)]
            fills += [(v_col[:, m, 0:1], kv_row[:, 128 * m:128 * (m + 1)])
                      for m in range(EXACT_M)]
            if kuv_d is not None:
                fills += [(kuv_sb[:, j, 0:1], kuv_row[:, 128 * j:128 * (j + 1)])
                          for j in range(J)]
                fills += [(kvf_col[:, m:m + 1], kvf_row[:, 128 * m:128 * (m + 1)])
                          for m in range(M)]
            for dst, src in fills:
                pt = pps.tile([128, 1], F32, tag="colfill", name="colfill")
                nc.tensor.transpose(pt, src, ones11)
                nc.vector.tensor_copy(dst, pt)
            for j in range(J):
                pt = pps.tile([128, BS], F32)
                nc.tensor.transpose(pt, in_nat[:, 128 * j:128 * (j + 1)],
                                    ident[0:BS, 0:BS])
                nc.vector.tensor_copy(xhT[:, j, :], pt)
            for j in range(M):
                pt = pps.tile([128, BS], F32)
                nc.tensor.transpose(pt, h_nat[:, 128 * j:128 * (j + 1)],
                                    ident[0:BS, 0:BS])
                nc.vector.tensor_copy(xhT[:, 2 * J + j, :], pt)
            for m in range(M):
                pwx = pps.tile([128, BS], F32)
                for j in range(M):
                    nc.tensor.matmul(pwx,
                                     lhsT=kw_sb[:, j, 128 * m:128 * (m + 1)],
                                     rhs=xhT[:, 2 * J + j, :],
                                     start=(j == 0), stop=(j == M - 1))
                nc.scalar.activation(bias_att[:, m, :], pwx, AF.Identity,
                                     bias=biasu_col[:, m:m + 1])
            if kuv_d is not None:
                # per-row constant of the linearized-tanh part:
                # c0[b] = sum_lin v_u * (Wx + bias_u)[u, b]
                ba16 = wpool.tile([128, M - EXACT_M, BS], BF16)
                nc.vector.tensor_copy(ba16, bias_att[:, EXACT_M:, :])
                c0_ps = pps.tile([1, BS], F32, name="c0ps")
                for idx, m in enumerate(range(EXACT_M, M)):
                    nc.tensor.matmul(c0_ps, lhsT=kvf_col[:, m:m + 1],
                                     rhs=ba16[:, idx, :],
                                     start=(idx == 0),
                                     stop=(idx == M - EXACT_M - 1))
                c0_row = wpool.tile([1, BS], F32)
                nc.vector.tensor_copy(c0_row, c0_ps)

        dump = cpool.tile([BS, U], F32)
        nc.vector.memset(dump, 0.0)
        if stage == "setup":
            nc.vector.tensor_copy(dump[:, 0:BS], xhT[0:BS, 0, :])
            nc.vector.tensor_copy(dump[:, BS:2 * BS], bias_att[0:BS, 0, :])
            nc.sync.dma_start(out=out_d, in_=dump)
            return

        # ------------- attention over the annotation stream -------------
        with (
            tc.tile_pool(name="ann", bufs=4) as annpool,
            tc.tile_pool(name="tanh", bufs=2) as tanhpool,
            tc.tile_pool(name="uh_ps", bufs=UHPS_BUFS, space="PSUM") as uhps,
            tc.tile_pool(name="et_ps", bufs=ETPS_BUFS, space="PSUM") as etps,
            tc.tile_pool(name="zp_ps", bufs=1, space="PSUM") as zpps,
            tc.tile_pool(name="small_sb", bufs=2) as smallsb,
            tc.tile_pool(name="wb_sb", bufs=2) as wbpool,
            tc.tile_pool(name="scr_sb", bufs=2) as scrpool,
        ):
            for b in range(BS):
                # x/h partial of one z-gate per odd b: overlaps the LSTM
                # matmuls with attention, leaving only ctx@W on the tail
                if stage == "full" and 1 <= b <= 4:
                    n = b - 1
                    nc.sync.dma_start(
                        out=Wn_t[n],
                        in_=WR_d[:, U * n:U * (n + 1)].rearrange(
                            "(k p) n -> p k n", p=128))
                if stage == "full" and b % 2 == 1:
                    n = (b - 1) // 2
                    zp = zpps.tile([BS, U], F32, tag="zp")
                    for idx, k in enumerate([0, 1, 2, 3, 8, 9, 10, 11]):
                        nc.tensor.matmul(zp, lhsT=xhT[:, k, :],
                                         rhs=Wn_t[n][:, k, :],
                                         start=(idx == 0), stop=False)
                    nc.tensor.matmul(zp, lhsT=ones1b,
                                     rhs=biasz_row[:, U * n:U * (n + 1)],
                                     start=False, stop=True)
                    if n == 2:
                        nc.vector.tensor_copy(z_part[:, n, :], zp)
                    else:
                        nc.vector.tensor_scalar(z_part_sc[:, n, :], zp,
                                                0.2, 0.5, op0=ALU.mult,
                                                op1=ALU.add)

                ctx_slot = smallsb.tile([128, J, NT], F32, tag="ctxslot")
                denb = smallsb.tile([1, NT], F32, tag="den")
                for i in range(NT):
                    annT = annpool.tile([128, J, TT], AT)
                    nc.sync.dma_start(
                        out=annT,
                        in_=annT_d[b, :, TT * i:TT * (i + 1)].rearrange(
                            "(j p) t -> p j t", p=128))
                    if annC_d is not None:
                        annC = annpool.tile([128, J, TT], BF16, tag="annC")
                        nc.sync.dma_start(
                            out=annC,
                            in_=annC_d[b, :, TT * i:TT * (i + 1)].rearrange(
                                "(j p) t -> p j t", p=128))
                    else:
                        annC = annT

                    # uh^T = ku^T @ ann^T, tanh(+Wx+bias_u) per exact m-chunk
                    tanhG = tanhpool.tile([128, EXACT_M, TT], ET)
                    for m in range(EXACT_M):
                        gps = uhps.tile([128, TH, 512], F32, tag="uh")
                        for th in range(TH):
                            if UH_FP8:
                                for g in range(J // 2):
                                    nc.tensor.matmul(
                                        gps[:, th, :],
                                        lhsT=ku_sb[:, 2 * g:2 * g + 2,
                                                   128 * m:128 * (m + 1)],
                                        rhs=annT[:, 2 * g:2 * g + 2,
                                                 512 * th:512 * (th + 1)],
                                        start=(g == 0), stop=(g == J // 2 - 1),
                                        perf_mode=PM.DoubleRow)
                            else:
                                for j in range(J):
                                    nc.tensor.matmul(
                                        gps[:, th, :],
                                        lhsT=ku_sb[:, j, 128 * m:128 * (m + 1)],
                                        rhs=annT[:, j, 512 * th:512 * (th + 1)],
                                        start=(j == 0), stop=(j == J - 1))
                        nc.scalar.activation(tanhG[:, m, :], gps, AF.Tanh,
                                             bias=bias_att[:, m, b:b + 1],
                                             scale=(1.0 / KU_SCALE
                                                    if UH_FP8 else 1.0))
                    if stage == "g":
                        nc.vector.tensor_copy(dump, tanhG[0:BS, 0, 0:U])
                        continue

                    # et = v . tanhG + kuv . ann^T  (PE), exp + den (scalar)
                    et_ps = etps.tile([1, TH, 512], F32, tag="et")
                    for th in range(TH):
                        ts = slice(512 * th, 512 * (th + 1))
                        mms = []
                        if ET_FP8:
                            for g in range(EXACT_M // 2):
                                mms.append((v_col[:, 2 * g:2 * g + 2, 0:1],
                                            tanhG[:, 2 * g:2 * g + 2, ts],
                                            PM.DoubleRow))
                        else:
                            for m in range(EXACT_M):
                                mms.append((v_col[:, m, 0:1],
                                            tanhG[:, m, ts], None))
                        if kuv_d is not None:
                            for g in range(J // 2):
                                mms.append((kuv_sb[:, 2 * g:2 * g + 2, 0:1],
                                            annT[:, 2 * g:2 * g + 2, ts],
                                            PM.DoubleRow))
                        for q, (lt, rh, pm) in enumerate(mms):
                            nc.tensor.matmul(et_ps[:, th, :], lhsT=lt, rhs=rh,
                                             start=(q == 0),
                                             stop=(q == len(mms) - 1),
                                             perf_mode=pm)
                    w_row = smallsb.tile([1, TT], BF16, tag="wrow")
                    nc.scalar.activation(w_row, et_ps, AF.Exp,
                                         scale=(1.0 / KU_SCALE
                                                if ET_FP8 else 1.0),
                                         bias=(c0_row[:, b:b + 1]
                                               if kuv_d is not None else 0.0),
                                         accum_out=denb[:, i:i + 1])
                    if stage == "et":
                        nc.vector.tensor_copy(dump[0:1, 0:U], w_row[:, 0:U])
                        continue

                    # context: broadcast softmax row, fused mul+reduce on DVE
                    wb = wbpool.tile([128, TT], BF16)
                    nc.gpsimd.partition_broadcast(wb, w_row)
                    scr = scrpool.tile([128, TT], BF16)
                    scr2 = scrpool.tile([128, TT], BF16, tag="scr2")
                    for j in range(J):
                        eng = (nc.gpsimd if j < CTX_ON_POOL else nc.vector)
                        eng.scalar_tensor_tensor(
                            out=(scr2 if j < CTX_ON_POOL else scr),
                            in0=annC[:, j, :], scalar=1.0, in1=wb,
                            op0=ALU.mult, op1=ALU.mult,
                            accum_out=ctx_slot[:, j, i:i + 1])

                if stage in ("g", "et"):
                    continue
                # normalize context into xhT[:, J:2J, b]
                dsum = smallsb.tile([1, 1], F32, tag="dsum")
                nc.vector.reduce_sum(dsum, denb, axis=mybir.AxisListType.X)
                drec = smallsb.tile([1, 1], F32, tag="drec")
                nc.vector.reciprocal(drec, dsum)
                drec_b = smallsb.tile([128, 1], F32, tag="drecb")
                nc.gpsimd.partition_broadcast(drec_b, drec)
                ctx_sum = smallsb.tile([128, J], F32, tag="ctxsum")
                nc.vector.tensor_add(ctx_sum, ctx_slot[:, :, 0],
                                     ctx_slot[:, :, 1])
                nc.vector.tensor_scalar_mul(xhT[:, J:2 * J, b:b + 1],
                                            ctx_sum, drec_b)
                if stage == "ctx":
                    nc.vector.tensor_copy(dump[0:1, 0:J], ctx_sum[0:1, :])

        if stage in ("g", "et", "ctx"):
            nc.sync.dma_start(out=out_d, in_=dump)
            return

        # ------------- LSTM tail: only ctx @ W remains here -------------
        with (
            tc.tile_pool(name="z_ps", bufs=1, space="PSUM") as zpool,
            tc.tile_pool(name="gates", bufs=1) as gpool,
        ):
            gates = [None] * 4
            for n in (2, 0, 1, 3):  # tanh gate first: ACT overlaps later MMs
                zps = zpool.tile([BS, U], F32, tag=f"zps{n}")
                for idx, k in enumerate(range(J, 2 * J)):
                    nc.tensor.matmul(zps, lhsT=xhT[:, k, :],
                                     rhs=Wn_t[n][:, k, :],
                                     start=(idx == 0), stop=(idx == J - 1))
                g = gpool.tile([BS, U], F32, tag=f"gate{n}")
                if n == 2:  # candidate cell state: tanh(z)
                    zsb = gpool.tile([BS, U], F32, tag="zsb2")
                    nc.vector.scalar_tensor_tensor(
                        out=zsb, in0=zps, scalar=0.0, in1=z_part[:, n, :],
                        op0=ALU.add, op1=ALU.add)
                    nc.scalar.activation(g, zsb, AF.Tanh)
                else:       # hard sigmoid: clip(0.2 z + 0.5, 0, 1) on DVE
                    zsb = gpool.tile([BS, U], F32, tag=f"zsb{n}")
                    nc.vector.scalar_tensor_tensor(
                        out=zsb, in0=zps, scalar=0.2,
                        in1=z_part_sc[:, n, :], op0=ALU.mult, op1=ALU.add)
                    nc.vector.tensor_scalar(g, zsb, 0.0, 1.0,
                                            op0=ALU.max, op1=ALU.min)
                gates[n] = g

            gi, gf, gg, go = gates
            c_new = gpool.tile([BS, U], F32, tag="cnew")
            nc.vector.tensor_mul(c_new, gf, c_nat)
            ig = gpool.tile([BS, U], F32, tag="ig")
            nc.vector.tensor_mul(ig, gi, gg)
            nc.vector.tensor_add(c_new, c_new, ig)
            tc_t = gpool.tile([BS, U], F32, tag="tanhc")
            nc.scalar.activation(tc_t, c_new, AF.Tanh)
            h_new = gpool.tile([BS, U], F32, tag="hnew")
            nc.vector.tensor_mul(h_new, go, tc_t)
            nc.sync.dma_start(out=out_d, in_=h_new)


_NC_CACHE = None


def _get_nc():
    global _NC_CACHE
    if _NC_CACHE is None:
        _NC_CACHE = build_bass()
    return _NC_CACHE


def make_in_maps(inputs, h, c, annotations, kernel, recurrent_kernel, bias,
                 kernel_u, kernel_w, kernel_v):
    asc = np.ascontiguousarray
    bf = ml_dtypes.bfloat16
    f8 = ml_dtypes.float8_e4m3
    at = f8 if UH_FP8 else bf
    wr = np.concatenate([np.asarray(kernel, np.float32),
                         np.asarray(recurrent_kernel, np.float32)],
                        axis=0).astype(bf)
    bias_f = asc(np.asarray(bias, np.float32)).reshape(1, 6 * U)
    # sort attention units by |v| so the small-|v| tail can use tanh(x)~=x
    kv_f = np.asarray(kernel_v, np.float32)
    perm = np.argsort(-np.abs(kv_f))
    EUv = 128 * EXACT_M
    ku_f = np.asarray(kernel_u, np.float32)[:, perm]
    kw_f = np.asarray(kernel_w, np.float32)[:, perm]
    kv_p = kv_f[perm]
    bup = asc(bias_f[0, 4 * U:5 * U][perm]).reshape(1, U)
    ku_q = (asc((ku_f[:, :EUv] * KU_SCALE).astype(f8)) if UH_FP8
            else asc(ku_f[:, :EUv].astype(bf)))
    kw16 = asc(kw_f.astype(bf))
    kv_q = asc((kv_p[:EUv] * (KU_SCALE if ET_FP8 else 1.0))
               .astype(np.float32)).reshape(1, EUv)
    lin_scale = KU_SCALE if ET_FP8 else 1.0
    kuv = asc((lin_scale * (ku_f[:, EUv:] @ kv_p[EUv:]))
              .astype(np.float32)).reshape(1, A)
    kvf = asc(kv_p.astype(np.float32)).reshape(1, U)
    maps = []
    for core in range(N_CORES):
        sl = slice(core * BS, (core + 1) * BS)
        annT_f = np.ascontiguousarray(
            np.asarray(annotations[sl], np.float32).transpose(0, 2, 1))
        m = {
            "annT": annT_f.astype(at),
            "inputs": asc(inputs[sl]).astype(np.float32),
            "h": asc(h[sl]).astype(np.float32),
            "c": asc(c[sl]).astype(np.float32),
            "wr": wr,
            "bias": bias_f,
            "ku": ku_q,
            "kw": kw16,
            "kv": kv_q,
            "bup": bup,
        }
        if EXACT_M < M:
            m["kuv"] = kuv
            m["kvf"] = kvf
        if UH_FP8 and not CTX_FP8:
            m["annC"] = annT_f.astype(bf)
        maps.append(m)
    return maps


def kernel(inputs, h, c, annotations, kernel, recurrent_kernel, bias,
           kernel_u, kernel_w, kernel_v, _trace=False):
    nc = _get_nc()
    in_maps = make_in_maps(inputs, h, c, annotations, kernel,
                           recurrent_kernel, bias, kernel_u, kernel_w,
                           kernel_v)
    res = run_bass_kernel_spmd(nc, in_maps, list(range(N_CORES)),
                               trace=_trace)
    out = np.concatenate([res.results[i]["out"] for i in range(N_CORES)],
                         axis=0)
    if _trace:
        kernel.last_exec_time_ns = res.exec_time_ns
        kernel.last_results = res
    return out


# revision 41
# speedup vs baseline: 1.2476x; 1.0211x over previous
"""Trainium2 Bass kernel for an attentive LSTM cell.

Data-parallel across 8 NeuronCores: batch (64) is sharded 8 per core, all
weights replicated.  Annotations are transposed and cast to bf16 on the
host, so the kernel streams ann^T [A, T] tiles that are directly usable as
the uh-matmul moving operand — no on-chip PE transposes, and half the HBM
traffic of fp32.

Per core, per batch row, for each [512, 1024] ann^T tile:
  1. uh^T = ku^T @ ann^T accumulated in PSUM (bf16 matmuls).
  2. tanh(uh + Wx + bias_u) on the scalar engine, N=1024 per instruction
     with the per-partition bias folded in.
  3. et = v . tanh(...) via v-stationary matmuls; exp on the scalar engine
     with the softmax denominator accumulated in the same instruction.
  4. softmax row broadcast to 128 partitions on gpsimd; context computed on
     the vector engine as a fused multiply+reduce over ann^T (contraction
     over t is the free dim in this layout), normalized at the end.

The LSTM tail (z = [x;h] @ [W;R] + b, gates, c/h update) runs batched over
the core's 8 rows with x^T/h^T assembled from tiny PE transposes; W and R
are concatenated and cast to bf16 on the host.
"""

import os
import sys

for _p in ("/opt/trn_rl_repo", "/root/.axon_site/_ro/trn_rl_repo"):
    if os.path.isdir(_p) and _p not in sys.path:
        sys.path.insert(0, _p)

import numpy as np
import ml_dtypes

import concourse.bass as bass
import concourse.mybir as mybir
import concourse.tile as tile
from concourse import bacc
from concourse.bass_utils import run_bass_kernel_spmd
from concourse.masks import make_identity

AF = mybir.ActivationFunctionType
ALU = mybir.AluOpType
F32 = mybir.dt.float32
BF16 = mybir.dt.bfloat16
FP8 = mybir.dt.float8e4
PM = mybir.MatmulPerfMode

UH_FP8 = True    # uh matmul in fp8 DoubleRow (ku prescaled x16 on host)
ET_FP8 = True    # tanh output + et matmul in fp8 DoubleRow
CTX_FP8 = True   # context DVE reduce reads the fp8 annotations (no bf16 copy)
KU_SCALE = 16.0
CTX_ON_POOL = 0  # scalar_tensor_tensor is not a valid Pool-engine op on HW
UHPS_BUFS = 2
ETPS_BUFS = 1
# et = v . tanh(uh + Wx): u-units are sorted by |v| on the host; only the
# EXACT_M top chunks get the tanh (and their uh matmul). For the small-|v|
# rest, tanh(x) ~= x, so their contribution collapses to a precomputed
# rank-1 vector kuv = ku_lin @ v_lin applied directly to ann^T, plus a
# per-row constant folded into the exp bias.
EXACT_M = 4

N_CORES = 8
B, T, A, U, D = 64, 2048, 512, 512, 512
BS = B // N_CORES   # batch rows per core
TT = 1024           # t macro-tile
NT = T // TT        # macro tiles per batch row
TH = TT // 512      # 512-col halves per macro tile (PSUM bank granularity)
J = A // 128        # contraction chunks (annotation dim)
M = U // 128        # unit chunks
KZ = (D + A + U) // 128  # contraction chunks for the z matmul ([x; h])


def build_bass(stage="full", repeat=1):
    nc = bacc.Bacc(trn_type="TRN2", debug=False)

    AT = FP8 if UH_FP8 else BF16
    EU = 128 * EXACT_M
    annT_d = nc.dram_tensor("annT", [BS, A, T], AT, kind="ExternalInput").ap()
    inp_d = nc.dram_tensor("inputs", [BS, D], F32, kind="ExternalInput").ap()
    h_d = nc.dram_tensor("h", [BS, U], F32, kind="ExternalInput").ap()
    c_d = nc.dram_tensor("c", [BS, U], F32, kind="ExternalInput").ap()
    WR_d = nc.dram_tensor("wr", [D + A + U, 4 * U], BF16, kind="ExternalInput").ap()
    bias_d = nc.dram_tensor("bias", [1, 6 * U], F32, kind="ExternalInput").ap()
    ku_d = nc.dram_tensor("ku", [A, EU], AT, kind="ExternalInput").ap()
    kw_d = nc.dram_tensor("kw", [U, U], BF16, kind="ExternalInput").ap()
    kv_d = nc.dram_tensor("kv", [1, EU], F32, kind="ExternalInput").ap()
    out_d = nc.dram_tensor("out", [BS, U], F32, kind="ExternalOutput").ap()
    bup_d = nc.dram_tensor("bup", [1, U], F32, kind="ExternalInput").ap()
    kuv_d = None
    kvf_d = None
    if EXACT_M < M:
        kuv_d = nc.dram_tensor("kuv", [1, A], F32, kind="ExternalInput").ap()
        kvf_d = nc.dram_tensor("kvf", [1, U], F32, kind="ExternalInput").ap()
    annC_d = None
    if not CTX_FP8 and UH_FP8:
        annC_d = nc.dram_tensor("annC", [BS, A, T], BF16,
                                kind="ExternalInput").ap()

    dd = dict(annT_d=annT_d, inp_d=inp_d, h_d=h_d, c_d=c_d, WR_d=WR_d,
              bias_d=bias_d, ku_d=ku_d, kw_d=kw_d, kv_d=kv_d, out_d=out_d,
              bup_d=bup_d, kuv_d=kuv_d, kvf_d=kvf_d, annC_d=annC_d)
    with tile.TileContext(nc) as tc:
        if repeat > 1:
            with tc.For_i(0, repeat, 1):
                _body(nc, tc, stage=stage, **dd)
        else:
            _body(nc, tc, stage=stage, **dd)
    nc.compile()
    return nc


def _body(nc, tc, annT_d, inp_d, h_d, c_d, WR_d, bias_d, ku_d, kw_d, kv_d,
          out_d, bup_d, kuv_d=None, kvf_d=None, annC_d=None, stage="full"):
    AT = FP8 if UH_FP8 else BF16
    ET = FP8 if ET_FP8 else BF16
    EU = 128 * EXACT_M
    with (
        tc.tile_pool(name="const", bufs=1) as cpool,
        tc.tile_pool(name="wts", bufs=1) as wpool,
    ):
        # touch the activation table set (exp/tanh share one) at t=0 so the
        # ~2.7us LoadActFuncSet runs before any real dependency chain
        warm = cpool.tile([1, 2], F32)
        nc.vector.memset(warm, 0.0)
        nc.scalar.activation(warm[:, 1:2], warm[:, 0:1], AF.Exp)

        ident = cpool.tile([128, 128], F32)
        make_identity(nc, ident)
        ones11 = cpool.tile([1, 1], F32)
        nc.vector.memset(ones11, 1.0)
        ones1b_ld = cpool.tile([1, BS], F32)
        nc.vector.memset(ones1b_ld, 1.0)
        ones1b = cpool.tile([1, BS], BF16)
        nc.vector.tensor_copy(ones1b, ones1b_ld)
        half_col = cpool.tile([BS, 1], F32)
        nc.vector.memset(half_col, 0.5)

        # --- replicated weights (already quantized in DRAM) ---
        ku_sb = wpool.tile([128, J, EU], AT)    # ku[a, u] -> [p, j, u]
        nc.sync.dma_start(out=ku_sb, in_=ku_d.rearrange("(j p) u -> p j u", p=128))
        kw_sb = wpool.tile([128, J, U], BF16)
        nc.sync.dma_start(out=kw_sb, in_=kw_d.rearrange("(j p) u -> p j u", p=128))
        # Small vectors are DMA'd as single contiguous rows and spread to
        # per-partition columns with tiny PE transposes: a rearrange-gather
        # DMA here would cost hundreds of 1-4 byte descriptors per repeat.
        if kuv_d is not None:
            # rank-1 linear-tanh weights; 16B-padded for DoubleRow LDWEIGHTS
            kuv_sb = wpool.tile([128, J, 16], AT)
            kuv_row = cpool.tile([1, A], F32)
            nc.sync.dma_start(out=kuv_row, in_=kuv_d)
            kvf_col = wpool.tile([128, M], BF16)   # full |v|-sorted v
            kvf_row = cpool.tile([1, U], F32)
            nc.sync.dma_start(out=kvf_row, in_=kvf_d)
        # z-matmul weights: all four gates resident up front (DMA triggered
        # at b==1 so the b=0 annotation tiles are queued first)
        Wn_t = [wpool.tile([128, KZ, U], BF16, tag=f"wn{n}", name=f"wn{n}")
                for n in range(4)]
        z_part = wpool.tile([BS, 4, U], F32)      # x/h part of z, gate 2 raw
        z_part_sc = wpool.tile([BS, 4, U], F32)   # 0.2*z_part+0.5 for i/f/o
        # v[u] -> [p, m, 0]; 16B-padded m-stride (fp8 DoubleRow LDWEIGHTS
        # requires the Ko-group step to be a multiple of 16 bytes)
        v_col = cpool.tile([128, EXACT_M, 16], ET)
        kv_row = cpool.tile([1, EU], F32)
        nc.sync.dma_start(out=kv_row, in_=kv_d)
        biasu_col = cpool.tile([128, M], F32)   # |v|-sorted bias_u column
        bup_row = cpool.tile([1, U], F32)
        nc.sync.dma_start(out=bup_row, in_=bup_d)
        biasz_ld = cpool.tile([1, 4 * U], F32)
        nc.sync.dma_start(out=biasz_ld, in_=bias_d[:, 0:4 * U])
        biasz_row = cpool.tile([1, 4 * U], BF16)
        nc.vector.tensor_copy(biasz_row, biasz_ld)

        # --- per-core state rows ---
        h_nat = cpool.tile([BS, U], F32)
        nc.sync.dma_start(out=h_nat, in_=h_d)
        in_nat = cpool.tile([BS, D], F32)
        nc.sync.dma_start(out=in_nat, in_=inp_d)
        c_nat = cpool.tile([BS, U], F32)
        nc.sync.dma_start(out=c_nat, in_=c_d)

        # [inputs; context; h]^T in contraction layout, bf16 for the z matmul
        xhT = wpool.tile([128, KZ, BS], BF16)
        bias_att = wpool.tile([128, M, BS], F32)  # Wx^T + bias_u per batch row

        with tc.tile_pool(name="ps_setup", bufs=2, space="PSUM") as pps:
            # spread the row-loaded vectors into per-partition columns
            fills = [(biasu_col[:, m:m + 1], bup_row[:, 128 * m:128 * (m + 1)])
                     for m in range(M)]
            fills += [(v_col[:, m, 0:1], kv_row[:, 128 * m:128 * (m + 1)])
                      for m in range(EXACT_M)]
            if kuv_d is not None:
                fills += [(kuv_sb[:, j, 0:1], kuv_row[:, 128 * j:128 * (j + 1)])
                          for j in range(J)]
                fills += [(kvf_col[:, m:m + 1], kvf_row[:, 128 * m:128 * (m + 1)])
                          for m in range(M)]
            for dst, src in fills:
                pt = pps.tile([128, 1], F32, tag="colfill", name="colfill")
                nc.tensor.transpose(pt, src, ones11)
                nc.vector.tensor_copy(dst, pt)
            for j in range(J):
                pt = pps.tile([128, BS], F32)
                nc.tensor.transpose(pt, in_nat[:, 128 * j:128 * (j + 1)],
                                    ident[0:BS, 0:BS])
                nc.vector.tensor_copy(xhT[:, j, :], pt)
            for j in range(M):
                pt = pps.tile([128, BS], F32)
                nc.tensor.transpose(pt, h_nat[:, 128 * j:128 * (j + 1)],
                                    ident[0:BS, 0:BS])
                nc.vector.tensor_copy(xhT[:, 2 * J + j, :], pt)
            for m in range(M):
                pwx = pps.tile([128, BS], F32)
                for j in range(M):
                    nc.tensor.matmul(pwx,
                                     lhsT=kw_sb[:, j, 128 * m:128 * (m + 1)],
                                     rhs=xhT[:, 2 * J + j, :],
                                     start=(j == 0), stop=(j == M - 1))
                nc.scalar.activation(bias_att[:, m, :], pwx, AF.Identity,
                                     bias=biasu_col[:, m:m + 1])
            if kuv_d is not None:
                # per-row constant of the linearized-tanh part:
                # c0[b] = sum_lin v_u * (Wx + bias_u)[u, b]
                ba16 = wpool.tile([128, M - EXACT_M, BS], BF16)
                nc.vector.tensor_copy(ba16, bias_att[:, EXACT_M:, :])
                c0_ps = pps.tile([1, BS], F32, name="c0ps")
                for idx, m in enumerate(range(EXACT_M, M)):
                    nc.tensor.matmul(c0_ps, lhsT=kvf_col[:, m:m + 1],
                                     rhs=ba16[:, idx, :],
                                     start=(idx == 0),
                                     stop=(idx == M - EXACT_M - 1))
                c0_row = wpool.tile([1, BS], F32)
                nc.vector.tensor_copy(c0_row, c0_ps)

        dump = cpool.tile([BS, U], F32)
        nc.vector.memset(dump, 0.0)
        if stage == "setup":
            nc.vector.tensor_copy(dump[:, 0:BS], xhT[0:BS, 0, :])
            nc.vector.tensor_copy(dump[:, BS:2 * BS], bias_att[0:BS, 0, :])
            nc.sync.dma_start(out=out_d, in_=dump)
            return

        # ------------- attention over the annotation stream -------------
        with (
            tc.tile_pool(name="ann", bufs=4) as annpool,
            tc.tile_pool(name="tanh", bufs=2) as tanhpool,
            tc.tile_pool(name="uh_ps", bufs=UHPS_BUFS, space="PSUM") as uhps,
            tc.tile_pool(name="et_ps", bufs=ETPS_BUFS, space="PSUM") as etps,
            tc.tile_pool(name="zp_ps", bufs=1, space="PSUM") as zpps,
            tc.tile_pool(name="small_sb", bufs=2) as smallsb,
            tc.tile_pool(name="wb_sb", bufs=2) as wbpool,
            tc.tile_pool(name="scr_sb", bufs=2) as scrpool,
        ):
            for b in range(BS):
                # x/h partial of one z-gate per odd b: overlaps the LSTM
                # matmuls with attention, leaving only ctx@W on the tail
                if stage == "full" and 1 <= b <= 4:
                    n = b - 1
                    nc.sync.dma_start(
                        out=Wn_t[n],
                        in_=WR_d[:, U * n:U * (n + 1)].rearrange(
                            "(k p) n -> p k n", p=128))
                if stage == "full" and b % 2 == 1:
                    n = (b - 1) // 2
                    zp = zpps.tile([BS, U], F32, tag="zp")
                    for idx, k in enumerate([0, 1, 2, 3, 8, 9, 10, 11]):
                        nc.tensor.matmul(zp, lhsT=xhT[:, k, :],
                                         rhs=Wn_t[n][:, k, :],
                                         start=(idx == 0), stop=False)
                    nc.tensor.matmul(zp, lhsT=ones1b,
                                     rhs=biasz_row[:, U * n:U * (n + 1)],
                                     start=False, stop=True)
                    if n == 2:
                        nc.vector.tensor_copy(z_part[:, n, :], zp)
                    else:
                        nc.vector.tensor_scalar(z_part_sc[:, n, :], zp,
                                                0.2, 0.5, op0=ALU.mult,
                                                op1=ALU.add)

                ctx_slot = smallsb.tile([128, J, NT], F32, tag="ctxslot")
                denb = smallsb.tile([1, NT], F32, tag="den")
                for i in range(NT):
                    annT = annpool.tile([128, J, TT], AT)
                    nc.sync.dma_start(
                        out=annT,
                        in_=annT_d[b, :, TT * i:TT * (i + 1)].rearrange(
                            "(j p) t -> p j t", p=128))
                    if annC_d is not None:
                        annC = annpool.tile([128, J, TT], BF16, tag="annC")
                        nc.sync.dma_start(
                            out=annC,
                            in_=annC_d[b, :, TT * i:TT * (i + 1)].rearrange(
                                "(j p) t -> p j t", p=128))
                    else:
                        annC = annT

                    # uh^T = ku^T @ ann^T, tanh(+Wx+bias_u) per exact m-chunk
                    tanhG = tanhpool.tile([128, EXACT_M, TT], ET)
                    for m in range(EXACT_M):
                        gps = uhps.tile([128, TH, 512], F32, tag="uh")
                        for th in range(TH):
                            if UH_FP8:
                                for g in range(J // 2):
                                    nc.tensor.matmul(
                                        gps[:, th, :],
                                        lhsT=ku_sb[:, 2 * g:2 * g + 2,
                                                   128 * m:128 * (m + 1)],
                                        rhs=annT[:, 2 * g:2 * g + 2,
                                                 512 * th:512 * (th + 1)],
                                        start=(g == 0), stop=(g == J // 2 - 1),
                                        perf_mode=PM.DoubleRow)
                            else:
                                for j in range(J):
                                    nc.tensor.matmul(
                                        gps[:, th, :],
                                        lhsT=ku_sb[:, j, 128 * m:128 * (m + 1)],
                                        rhs=annT[:, j, 512 * th:512 * (th + 1)],
                                        start=(j == 0), stop=(j == J - 1))
                        nc.scalar.activation(tanhG[:, m, :], gps, AF.Tanh,
                                             bias=bias_att[:, m, b:b + 1],
                                             scale=(1.0 / KU_SCALE
                                                    if UH_FP8 else 1.0))
                    if stage == "g":
                        nc.vector.tensor_copy(dump, tanhG[0:BS, 0, 0:U])
                        continue

                    # et = v . tanhG + kuv . ann^T  (PE), exp + den (scalar)
                    et_ps = etps.tile([1, TH, 512], F32, tag="et")
                    for th in range(TH):
                        ts = slice(512 * th, 512 * (th + 1))
                        mms = []
                        if ET_FP8:
                            for g in range(EXACT_M // 2):
                                mms.append((v_col[:, 2 * g:2 * g + 2, 0:1],
                                            tanhG[:, 2 * g:2 * g + 2, ts],
                                            PM.DoubleRow))
                        else:
                            for m in range(EXACT_M):
                                mms.append((v_col[:, m, 0:1],
                                            tanhG[:, m, ts], None))
                        if kuv_d is not None:
                            for g in range(J // 2):
                                mms.append((kuv_sb[:, 2 * g:2 * g + 2, 0:1],
                                            annT[:, 2 * g:2 * g + 2, ts],
                                            PM.DoubleRow))
                        for q, (lt, rh, pm) in enumerate(mms):
                            nc.tensor.matmul(et_ps[:, th, :], lhsT=lt, rhs=rh,
                                             start=(q == 0),
                                             stop=(q == len(mms) - 1),
                                             perf_mode=pm)
                    w_row = smallsb.tile([1, TT], BF16, tag="wrow")
                    nc.scalar.activation(w_row, et_ps, AF.Exp,
                                         scale=(1.0 / KU_SCALE
                                                if ET_FP8 else 1.0),
                                         bias=(c0_row[:, b:b + 1]
                                               if kuv_d is not None else 0.0),
                                         accum_out=denb[:, i:i + 1])
                    if stage == "et":
                        nc.vector.tensor_copy(dump[0:1, 0:U], w_row[:, 0:U])
                        continue

                    # context: broadcast softmax row, fused mul+reduce on DVE
                    wb = wbpool.tile([128, TT], BF16)
                    nc.gpsimd.partition_broadcast(wb, w_row)
                    scr = scrpool.tile([128, TT], BF16)
                    scr2 = scrpool.tile([128, TT], BF16, tag="scr2")
                    for j in range(J):
                        eng = (nc.gpsimd if j < CTX_ON_POOL else nc.vector)
                        eng.scalar_tensor_tensor(
                            out=(scr2 if j < CTX_ON_POOL else scr),
                            in0=annC[:, j, :], scalar=1.0, in1=wb,
                            op0=ALU.mult, op1=ALU.mult,
                            accum_out=ctx_slot[:, j, i:i + 1])

                if stage in ("g", "et"):
                    continue
                # normalize context into xhT[:, J:2J, b]
                dsum = smallsb.tile([1, 1], F32, tag="dsum")
                nc.vector.reduce_sum(dsum, denb, axis=mybir.AxisListType.X)
                drec = smallsb.tile([1, 1], F32, tag="drec")
                nc.vector.reciprocal(drec, dsum)
                drec_b = smallsb.tile([128, 1], F32, tag="drecb")
                nc.gpsimd.partition_broadcast(drec_b, drec)
                ctx_sum = smallsb.tile([128, J], F32, tag="ctxsum")
                nc.vector.tensor_add(ctx_sum, ctx_slot[:, :, 0],
                                     ctx_slot[:, :, 1])
                nc.vector.tensor_scalar_mul(xhT[:, J:2 * J, b:b + 1],
                                            ctx_sum, drec_b)
                if stage == "ctx":
                    nc.vector.tensor_copy(dump[0:1, 0:J], ctx_sum[0:1, :])

        if stage in ("g", "et", "ctx"):
            nc.sync.dma_start(out=out_d, in_=dump)
            return

        # ------------- LSTM tail: only ctx @ W remains here -------------
        with (
            tc.tile_pool(name="z_ps", bufs=1, space="PSUM") as zpool,
            tc.tile_pool(name="gates", bufs=1) as gpool,
        ):
            gates = [None] * 4
            for n in (2, 0, 1, 3):  # tanh gate first: ACT overlaps later MMs
                zps = zpool.tile([BS, U], F32, tag=f"zps{n}")
                for idx, k in enumerate(range(J, 2 * J)):
                    nc.tensor.matmul(zps, lhsT=xhT[:, k, :],
                                     rhs=Wn_t[n][:, k, :],
                                     start=(idx == 0), stop=(idx == J - 1))
                g = gpool.tile([BS, U], F32, tag=f"gate{n}")
                if n == 2:  # candidate cell state: tanh(z)
                    zsb = gpool.tile([BS, U], F32, tag="zsb2")
                    nc.vector.scalar_tensor_tensor(
                        out=zsb, in0=zps, scalar=0.0, in1=z_part[:, n, :],
                        op0=ALU.add, op1=ALU.add)
                    nc.scalar.activation(g, zsb, AF.Tanh)
                else:       # hard sigmoid: clip(0.2 z + 0.5, 0, 1) on DVE
                    zsb = gpool.tile([BS, U], F32, tag=f"zsb{n}")
                    nc.vector.scalar_tensor_tensor(
                        out=zsb, in0=zps, scalar=0.2,
                        in1=z_part_sc[:, n, :], op0=ALU.mult, op1=ALU.add)
                    nc.vector.tensor_scalar(g, zsb, 0.0, 1.0,
                                            op0=ALU.max, op1=ALU.min)
                gates[n] = g

            gi, gf, gg, go = gates
            c_new = gpool.tile([BS, U], F32, tag="cnew")
            nc.vector.tensor_mul(c_new, gf, c_nat)
            ig = gpool.tile([BS, U], F32, tag="ig")
            nc.vector.tensor_mul(ig, gi, gg)
            nc.vector.tensor_add(c_new, c_new, ig)
            tc_t = gpool.tile([BS, U], F32, tag="tanhc")
            nc.scalar.activation(tc_t, c_new, AF.Tanh)
            h_new = gpool.tile([BS, U], F32, tag="hnew")
            nc.vector.tensor_mul(h_new, go, tc_t)
            nc.sync.dma_start(out=out_d, in_=h_new)


_NC_CACHE = None


def _get_nc():
    global _NC_CACHE
    if _NC_CACHE is None:
        _NC_CACHE = build_bass()
    return _NC_CACHE


def make_in_maps(inputs, h, c, annotations, kernel, recurrent_kernel, bias,
                 kernel_u, kernel_w, kernel_v):
    asc = np.ascontiguousarray
    bf = ml_dtypes.bfloat16
    f8 = ml_dtypes.float8_e4m3
    at = f8 if UH_FP8 else bf
    wr = np.concatenate([np.asarray(kernel, np.float32),
                         np.asarray(recurrent_kernel, np.float32)],
                        axis=0).astype(bf)
    bias_f = asc(np.asarray(bias, np.float32)).reshape(1, 6 * U)
    # sort attention units by |v| so the small-|v| tail can use tanh(x)~=x
    kv_f = np.asarray(kernel_v, np.float32)
    perm = np.argsort(-np.abs(kv_f))
    EUv = 128 * EXACT_M
    ku_f = np.asarray(kernel_u, np.float32)[:, perm]
    kw_f = np.asarray(kernel_w, np.float32)[:, perm]
    kv_p = kv_f[perm]
    bup = asc(bias_f[0, 4 * U:5 * U][perm]).reshape(1, U)
    ku_q = (asc((ku_f[:, :EUv] * KU_SCALE).astype(f8)) if UH_FP8
            else asc(ku_f[:, :EUv].astype(bf)))
    kw16 = asc(kw_f.astype(bf))
    kv_q = asc((kv_p[:EUv] * (KU_SCALE if ET_FP8 else 1.0))
               .astype(np.float32)).reshape(1, EUv)
    lin_scale = KU_SCALE if ET_FP8 else 1.0
    kuv = asc((lin_scale * (ku_f[:, EUv:] @ kv_p[EUv:]))
              .astype(np.float32)).reshape(1, A)
    kvf = asc(kv_p.astype(np.float32)).reshape(1, U)
    maps = []
    for core in range(N_CORES):
        sl = slice(core * BS, (core + 1) * BS)
        annT_f = np.ascontiguousarray(
            np.asarray(annotations[sl], np.float32).transpose(0, 2, 1))
        m = {
            "annT": annT_f.astype(at),
            "inputs": asc(inputs[sl]).astype(np.float32),
            "h": asc(h[sl]).astype(np.float32),
            "c": asc(c[sl]).astype(np.float32),
            "wr": wr,
            "bias": bias_f,
            "ku": ku_q,
            "kw": kw16,
            "kv": kv_q,
            "bup": bup,
        }
        if EXACT_M < M:
            m["kuv"] = kuv
            m["kvf"] = kvf
        if UH_FP8 and not CTX_FP8:
            m["annC"] = annT_f.astype(bf)
        maps.append(m)
    return maps


def kernel(inputs, h, c, annotations, kernel, recurrent_kernel, bias,
           kernel_u, kernel_w, kernel_v, _trace=False):
    nc = _get_nc()
    in_maps = make_in_maps(inputs, h, c, annotations, kernel,
                           recurrent_kernel, bias, kernel_u, kernel_w,
                           kernel_v)
    res = run_bass_kernel_spmd(nc, in_maps, list(range(N_CORES)),
                               trace=_trace)
    out = np.concatenate([res.results[i]["out"] for i in range(N_CORES)],
                         axis=0)
    if _trace:
        kernel.last_exec_time_ns = res.exec_time_ns
        kernel.last_results = res
    return out
